# revision 1
# baseline (speedup 1.0000x reference)
"""Trainium2 Bass kernel for XCA-style attention block (nn_Attention_4612794876668).

Sharding: data-parallel over batch (B=8) across 8 NeuronCores; params replicated.
Per-core layout: channels on partitions, flattened spatial (H*W) on the free dim.
 - 1x1 convs  -> TensorE matmuls (bf16 operands, fp32 PSUM accumulate)
 - depthwise 3x3 -> 9 accumulating diagonal matmuls with spatially-shifted APs
 - q/k Gram (channel attention is over a 24x24 per-head matrix) -> PE transposes
   + per-head Gram accumulation in a persistent PSUM bank
 - softmax / norms on small [<=48, 384] tensors via DVE/ACT
 - attn@v -> block-diagonal matmul; proj -> matmul; pos branch -> two more
   diagonal-matmul depthwise convs with exact gelu between.
"""

import sys

sys.path.insert(0, "/opt/trn_rl_repo")

import numpy as np
import ml_dtypes

import concourse.bass as bass
import concourse.mybir as mybir
import concourse.tile as tile
from concourse import bacc
from concourse.bass_utils import run_bass_kernel_spmd
from concourse.masks import make_identity

F32 = mybir.dt.float32
FP8 = mybir.dt.float8e4
BF16 = mybir.dt.bfloat16
AF = mybir.ActivationFunctionType
ALU = mybir.AluOpType

B, C, H, W = 8, 192, 128, 128
C3 = 3 * C                      # 576
HEADS, CH = 8, 24
TH = 16                         # image rows per spatial tile
NT = H // TH                    # 8 spatial tiles
# channel chunks
CH3 = [(0, 128), (128, 256), (256, 384), (384, 512), (512, 576)]   # qkv space
CHC = [(0, 128), (128, 192)]                                       # 192 space
TAPS = [(i, j) for i in range(3) for j in range(3)]


DW_SCALE = 1.0


def _evac(nc, idx, out_ap, in_ap, bias=None, scale=1.0):
    """PSUM -> SBUF evacuation alternating between ACT and DVE."""
    if idx % 2 == 0:
        if bias is None and scale == 1.0:
            nc.scalar.copy(out_ap, in_ap)
        else:
            nc.scalar.activation(out_ap, in_ap, AF.Identity,
                                 bias=0.0 if bias is None else bias, scale=scale)
    else:
        if bias is None and scale == 1.0:
            nc.vector.tensor_copy(out_ap, in_ap)
        elif scale == 1.0:
            nc.vector.tensor_scalar_add(out_ap, in_ap, bias)
        else:
            nc.vector.tensor_scalar(out_ap, in_ap, scale,
                                    0.0 if bias is None else bias,
                                    ALU.mult, ALU.add)


def _dw_dr_matmuls(nc, psum3, dq8_sb, m, qp_tile, row_off, nrows):
    """fp8 DoubleRow depthwise conv: 6 passes (3 vertical pairs + 3 singles).

    pass p in 0..2: dx = p-1, taps (dy=-1, dy=0) paired, g-step = +W rows.
    pass p in 3..5: dx = p-4, tap dy=+1 with zero second weight, g-step = -W.
    """
    ap0 = qp_tile[:]
    pstep = ap0.ap[0][0]
    for p in range(6):
        if p < 3:
            dx = p - 1
            dy0, gstep = -1, W
        else:
            dx = p - 4
            dy0, gstep = 1, -W
        r0 = row_off + dy0
        if dx == -1:
            o = psum3[:, 0:nrows, 1:128]
            c0, ncols = 0, 127
        elif dx == 0:
            o = psum3[:, 0:nrows, :]
            c0, ncols = 0, 128
        else:
            o = psum3[:, 0:nrows, 0:127]
            c0, ncols = 1, 127
        off = ap0.offset + r0 * W + c0
        rhs = bass.AP(ap0.tensor, off,
                      [[pstep, 128], [gstep, 2], [W, nrows], [1, ncols]])
        nc.tensor.matmul(o, dq8_sb[:, m, p, :, :], rhs,
                         perf_mode=mybir.MatmulPerfMode.DoubleRow,
                         start=(p == 0), stop=(p == 5))


def _dw_matmuls(nc, psum3, lhsT_taps, src3, row_off, nrows, start_extra=True):
    """Accumulate a 3x3 depthwise conv tap set into psum3 [P, nrows, 128].

    src3: SBUF tile AP [P, R, 128]; output rows r map to src rows r+row_off+i-1.
    """
    for tap, (i, j) in enumerate(TAPS):
        st = (tap == 0) and start_extra
        sp = tap == len(TAPS) - 1
        r0 = row_off + i - 1
        if j == 0:
            o = psum3[:, 0:nrows, 1:128]
            s = src3[:, r0 : r0 + nrows, 0:127]
        elif j == 1:
            o = psum3[:, 0:nrows, :]
            s = src3[:, r0 : r0 + nrows, :]
        else:
            o = psum3[:, 0:nrows, 0:127]
            s = src3[:, r0 : r0 + nrows, 1:128]
        nc.tensor.matmul(o, lhsT_taps[tap], s, start=st, stop=sp)


def build_kernel():
    nc = bacc.Bacc(None, target_bir_lowering=False)

    # ---- DRAM parameters (per-core) ----
    x_d = nc.declare_dram_parameter("x", [C, H, W], BF16, isOutput=False)
    wqkvT_d = nc.declare_dram_parameter("wqkvT", [C, C3], BF16, isOutput=False)
    wprojT_d = nc.declare_dram_parameter("wprojT", [C, C], BF16, isOutput=False)
    dqkv_d = nc.declare_dram_parameter("dqkv", [128, 2, 9, 128], BF16, isOutput=False)
    dq8_d = nc.declare_dram_parameter("dq8", [128, 3, 6, 2, 128], FP8, isOutput=False)
    dpos1_d = nc.declare_dram_parameter("dpos1", [128, 2, 9, 128], BF16, isOutput=False)
    dpos2_d = nc.declare_dram_parameter("dpos2", [128, 2, 9, 128], BF16, isOutput=False)
    bqkv_d = nc.declare_dram_parameter("bqkv", [128, 5], F32, isOutput=False)
    bdw_d = nc.declare_dram_parameter("bdw", [128, 5], F32, isOutput=False)
    bproj_d = nc.declare_dram_parameter("bproj", [128, 2], F32, isOutput=False)
    temp_d = nc.declare_dram_parameter("temp", [8, 1], F32, isOutput=False)
    idmaskA_d = nc.declare_dram_parameter("idmaskA", [96, 384], F32, isOutput=False)
    idmaskB_d = nc.declare_dram_parameter("idmaskB", [96, 192], F32, isOutput=False)
    out_d = nc.declare_dram_parameter("out", [C, H, W], F32, isOutput=True)

    v_hbm = nc.dram_tensor("v_hbm", [C, H, W], BF16)

    with tile.TileContext(nc) as tc:
        with (
            tc.tile_pool(name="const", bufs=1) as cp,
            tc.tile_pool(name="work", bufs=2) as wp,
            tc.tile_pool(name="small", bufs=1) as sp,
            tc.tile_pool(name="one", bufs=1) as op,
            tc.tile_pool(name="ps", bufs=2, space="PSUM") as ps,
            tc.tile_pool(name="ps3", bufs=3, space="PSUM") as ps3,
            tc.tile_pool(name="psg", bufs=1, space="PSUM") as psg,
        ):
            # ---- load constants ----
            wq_sb = [cp.tile([128, C3], BF16, tag="wq0", name="wq0"), cp.tile([64, C3], BF16, tag="wq1", name="wq1")]
            nc.sync.dma_start(wq_sb[0][:], wqkvT_d[0:128])
            nc.sync.dma_start(wq_sb[1][:], wqkvT_d[128:192])
            wp_sb = [cp.tile([128, C], BF16, tag="wp0", name="wp0"), cp.tile([64, C], BF16, tag="wp1", name="wp1")]
            nc.sync.dma_start(wp_sb[0][:], wprojT_d[0:128])
            nc.sync.dma_start(wp_sb[1][:], wprojT_d[128:192])
            dq_sb = cp.tile([128, 2, 9, 128], BF16, tag="dq", name="dq")
            nc.sync.dma_start(dq_sb[:], dqkv_d[:])
            dq8_sb = cp.tile([128, 3, 6, 2, 128], FP8, tag="dq8", name="dq8")
            nc.sync.dma_start(dq8_sb[:], dq8_d[:])
            dp1_sb = cp.tile([128, 2, 9, 128], BF16, tag="dp1", name="dp1")
            nc.sync.dma_start(dp1_sb[:], dpos1_d[:])
            dp2_sb = cp.tile([128, 2, 9, 128], BF16, tag="dp2", name="dp2")
            nc.sync.dma_start(dp2_sb[:], dpos2_d[:])
            bqkv_sb = cp.tile([128, 5], F32, tag="bqkv", name="bqkv")
            nc.sync.dma_start(bqkv_sb[:], bqkv_d[:])
            bdw_sb = cp.tile([128, 5], F32, tag="bdw", name="bdw")
            nc.sync.dma_start(bdw_sb[:], bdw_d[:])
            bproj_sb = cp.tile([128, 2], F32, tag="bproj", name="bproj")
            nc.sync.dma_start(bproj_sb[:], bproj_d[:])
            temp_sb = cp.tile([8, 1], F32, tag="temp", name="temp")
            nc.sync.dma_start(temp_sb[:], temp_d[:])
            idmA_sb = cp.tile([96, 384], F32, tag="idmA", name="idmA")
            nc.sync.dma_start(idmA_sb[:], idmaskA_d[:])
            idmB_sb = cp.tile([96, 192], F32, tag="idmB", name="idmB")
            nc.sync.dma_start(idmB_sb[:], idmaskB_d[:])
            ones8 = cp.tile([8, 24], BF16, tag="ones8", name="ones8")
            nc.gpsimd.memset(ones8[:], 1.0)

            ident_bf = cp.tile([128, 128], BF16, tag="idb", name="idb")
            make_identity(nc, ident_bf[:])
            ident_f = cp.tile([128, 128], F32, tag="idf", name="idf")
            make_identity(nc, ident_f[:])

            # persistent Gram accumulators: q-rows x [q|k] and k-rows x k
            g_ps = psg.tile([96, 384], F32, tag="gram", name="gram")
            gkk_ps = psg.tile([96, 192], F32, tag="gram2", name="gram2")

            CSZ3 = [c1 - c0 for (c0, c1) in CH3]
            CSZC = [c1 - c0 for (c0, c1) in CHC]

            # =================== PHASE 1 ===================
            for t in range(NT):
                r0 = t * TH
                # x tile with 1-row halo: rows r0-1 .. r0+16  (18 rows)
                xt = [
                    wp.tile([128, 18, W], BF16, tag="xt0", name="xt0"),
                    wp.tile([64, 18, W], BF16, tag="xt1", name="xt1"),
                ]
                for k, (c0, c1) in enumerate(CHC):
                    if t == 0:
                        nc.gpsimd.memset(xt[k][:, 0:1, :], 0.0)
                        nc.sync.dma_start(xt[k][:, 1:18, :], x_d[c0:c1, 0:17, :])
                    elif t == NT - 1:
                        nc.gpsimd.memset(xt[k][:, 17:18, :], 0.0)
                        nc.sync.dma_start(xt[k][:, 0:17, :], x_d[c0:c1, r0 - 1 : 128, :])
                    else:
                        nc.sync.dma_start(xt[k][:], x_d[c0:c1, r0 - 1 : r0 + 17, :])

                # ---- qkv_pre = wqkv @ x (+bqkv) over 18 rows ----
                NROWS_PRE = [(0, 4), (4, 8), (8, 12), (12, 16), (16, 18)]
                qp = [wp.tile([CSZ3[m], 18, W], FP8 if m < 3 else BF16, tag=f"qp{m}", name=f"qp{m}") for m in range(5)]
                ei = 0
                for m, (o0, o1) in enumerate(CH3):
                    for (a, b) in NROWS_PRE:
                        pre_ps = ps3.tile([128, 512], F32, tag="pre", name="pre")
                        o = pre_ps[: CSZ3[m], 0 : (b - a) * W].rearrange(
                            "p (r w) -> p r w", w=W
                        )
                        for k in range(2):
                            nc.tensor.matmul(
                                o,
                                wq_sb[k][:, o0:o1],
                                xt[k][:, a:b, :],
                                start=(k == 0),
                                stop=(k == 1),
                            )
                        _evac(nc, ei, qp[m][:, a:b, :], o, bias=bqkv_sb[:CSZ3[m], m : m + 1])
                        ei += 1
                # zero the out-of-image halo row (dwconv pads with 0, not bias)
                if t == 0:
                    for m in range(5):
                        nc.gpsimd.memset(qp[m][:, 0:1, :], 0.0)
                if t == NT - 1:
                    for m in range(5):
                        nc.gpsimd.memset(qp[m][:, 17:18, :], 0.0)

                # ---- qkv_dw: depthwise 3x3 on qp -> 16 valid rows ----
                qkd = [wp.tile([CSZ3[m], TH, W], BF16, tag=f"qkd{m}", name=f"qkd{m}") for m in range(3)]
                vt_out = [wp.tile([CSZ3[3 + k], TH, W], BF16, tag=f"vt{k}", name=f"vt{k}") for k in range(2)]
                ei = 0
                for m in range(5):
                    msz = CSZ3[m]
                    for nn in range(4):
                        dw_ps = ps3.tile([128, 512], F32, tag="dw", name="dw")
                        p3 = dw_ps[:msz].rearrange("p (r w) -> p r w", w=W)
                        if m < 3:
                            _dw_dr_matmuls(nc, p3, dq8_sb, m, qp[m], row_off=1 + 4 * nn, nrows=4)
                            _evac(nc, ei, qkd[m][:, 4 * nn : 4 * nn + 4, :], p3,
                                  bias=bdw_sb[:msz, m : m + 1])
                        else:
                            lhsT_taps = [dq_sb[:msz, m - 3, tap, :msz] for tap in range(9)]
                            _dw_matmuls(nc, p3, lhsT_taps, qp[m][:], row_off=1 + 4 * nn, nrows=4)
                            _evac(nc, ei, vt_out[m - 3][:, 4 * nn : 4 * nn + 4, :], p3,
                                  bias=bdw_sb[:msz, m : m + 1])
                        ei += 1
                for k, (c0, c1) in enumerate([(384, 512), (512, 576)]):
                    nc.sync.dma_start(
                        v_hbm[c0 - 384 : c1 - 384, r0 : r0 + TH, :], vt_out[k][:]
                    )

                # ---- transposes of q,k (bf16) + per-head Gram accumulation ----
                zt = op.tile([128, TH, 384], BF16, tag="zt", name="zt")
                for m in range(3):
                    for b4 in range(TH // 4):
                        tp_ps = ps3.tile([128, 4, 128], BF16, tag="pre", name="tp")
                        for i in range(4):
                            nc.tensor.matmul(
                                tp_ps[:, i, :], qkd[m][:, 4 * b4 + i, :], ident_bf[:],
                                is_transpose=True, start=(i == 0), stop=(i == 3),
                                skip_group_check=True,
                            )
                        nc.vector.tensor_copy(
                            zt[:, 4 * b4 : 4 * b4 + 4, 128 * m : 128 * (m + 1)], tp_ps[:]
                        )
                ztv = zt[:].rearrange("p r (g c) -> p r g c", g=2)
                for bb in range(TH):
                    for q in range(2):
                        first = bool(t == 0 and bb == 0 and q == 0)
                        last = bool(t == NT - 1 and bb == TH - 1 and q == 1)
                        qsl = ztv[:, bb, 0, 96 * q : 96 * (q + 1)]
                        ksl = ztv[:, bb, 1, 96 * q : 96 * (q + 1)]
                        zsl = ztv[:, bb, :, 96 * q : 96 * (q + 1)]
                        nc.tensor.matmul(
                            g_ps[:, 192 * q : 192 * (q + 1)], qsl, zsl,
                            start=first, stop=last, skip_group_check=True,
                        )
                        nc.tensor.matmul(
                            gkk_ps[:, 96 * q : 96 * (q + 1)], ksl, ksl,
                            start=first, stop=last, skip_group_check=True,
                        )

            # =================== PHASE 2a: attention matrices ===================
            g_sb = sp.tile([96, 384], F32, tag="gsb", name="gsb")
            nc.vector.tensor_copy(g_sb[:], g_ps[:])
            gkk_sb = sp.tile([96, 192], F32, tag="gkksb", name="gkksb")
            nc.vector.tensor_copy(gkk_sb[:], gkk_ps[:])
            # squared norms via mask-and-reduce (diag extraction), quad layout
            mq = sp.tile([96, 384], F32, tag="mq", name="mq")
            nc.vector.tensor_tensor(mq[:], g_sb[:], idmA_sb[:], ALU.mult)
            nq96 = sp.tile([96, 2], F32, tag="nq96", name="nq96")
            nc.vector.tensor_reduce(
                nq96[:], mq[:].rearrange("p (q c) -> p q c", q=2),
                axis=mybir.AxisListType.X, op=ALU.add,
            )
            mk = sp.tile([96, 192], F32, tag="mk", name="mk")
            nc.vector.tensor_tensor(mk[:], gkk_sb[:], idmB_sb[:], ALU.mult)
            nk96 = sp.tile([96, 2], F32, tag="nk96", name="nk96")
            nc.vector.tensor_reduce(
                nk96[:], mk[:].rearrange("p (q c) -> p q c", q=2),
                axis=mybir.AxisListType.X, op=ALU.add,
            )
            # gather into [24, 16] (cols: 8 q-heads then 8 k-heads) + S into [24, 192]
            n2 = sp.tile([24, 16], F32, tag="n2", name="n2")
            s_all = sp.tile([24, 192], F32, tag="sall", name="sall")
            for h in range(8):
                hp, q = 24 * (h % 4), h // 4
                nc.sync.dma_start(n2[:, h : h + 1], nq96[hp : hp + 24, q : q + 1])
                nc.sync.dma_start(n2[:, 8 + h : 9 + h], nk96[hp : hp + 24, q : q + 1])
                nc.sync.dma_start(
                    s_all[:, CH * h : CH * (h + 1)],
                    g_sb[hp : hp + 24, 192 * q + 96 + hp : 192 * q + 96 + hp + 24],
                )
            # rn = 1 / max(sqrt(n2), eps) for both q and k sides: [24, 16]
            nrm = sp.tile([24, 16], F32, tag="nrm", name="nrm")
            nc.scalar.sqrt(nrm[:], n2[:])
            nc.vector.tensor_scalar_max(nrm[:], nrm[:], 1e-12)
            rn = sp.tile([24, 16], F32, tag="rn", name="rn")
            nc.vector.reciprocal(rn[:], nrm[:])
            rnq = rn[:, 0:8]                      # [24, 8] f32, per-partition q scales
            rnk_bf = sp.tile([24, 8], BF16, tag="rnkbf", name="rnkbf")
            nc.vector.tensor_copy(rnk_bf[:], rn[:, 8:16])
            # transpose k-scales -> [8, 24], fold temperature, build block-diag bcast
            rnt_ps = psg.tile([8, 24], BF16, tag="gram2", name="rnt")
            nc.tensor.transpose(rnt_ps[:], rnk_bf[:], ident_bf[:24, :24])
            rnkT = sp.tile([8, 24], BF16, tag="rnkT", name="rnkT")
            nc.vector.tensor_copy(rnkT[:], rnt_ps[:])
            nc.vector.tensor_scalar_mul(rnkT[:], rnkT[:], temp_sb[:, 0:1])
            kdiag = sp.tile([8, 8, 24], BF16, tag="kdiag", name="kdiag")
            nc.vector.tensor_copy(
                kdiag[:], rnkT[:].unsqueeze(1).to_broadcast((8, 8, 24))
            )
            nc.gpsimd.affine_select(
                out=kdiag[:].rearrange("p a b -> p (a b)"),
                in_=kdiag[:].rearrange("p a b -> p (a b)"),
                compare_op=ALU.is_equal, fill=0.0,
                base=0, pattern=[[-1, 8], [0, 24]], channel_multiplier=1,
            )
            # Rk[c, (h,d)] = rn_k[d,h]*tau_h, replicated across partitions c
            rk_ps = psg.tile([24, 192], F32, tag="gram2", name="rkps")
            nc.tensor.matmul(
                rk_ps[:], ones8[:],
                kdiag[:].rearrange("p a b -> p (a b)"),
                start=True, stop=True,
            )
            # logits = S * Rk * rn_q
            sview = s_all[:].rearrange("p (h c) -> p h c", h=8)
            lg = sp.tile([24, 192], F32, tag="lg", name="lg")
            nc.vector.tensor_tensor(
                lg[:].rearrange("p (h c) -> p h c", h=8), sview,
                rk_ps[:].rearrange("p (h c) -> p h c", h=8), ALU.mult,
            )
            nc.vector.tensor_tensor(
                lg[:].rearrange("p (h c) -> p h c", h=8),
                lg[:].rearrange("p (h c) -> p h c", h=8),
                rnq[:, :, None].to_broadcast((24, 8, 24)), ALU.mult,
            )
            # softmax over last dim (logits are in [-tau, tau], no max-sub needed)
            ex = sp.tile([24, 192], F32, tag="ex", name="ex")
            nc.scalar.activation(ex[:], lg[:], AF.Exp)
            rs = sp.tile([24, 8], F32, tag="rs", name="rs")
            nc.vector.tensor_reduce(
                rs[:], ex[:].rearrange("p (h c) -> p h c", h=8),
                axis=mybir.AxisListType.X, op=ALU.add,
            )
            rr = sp.tile([24, 8], F32, tag="rr", name="rr")
            nc.vector.reciprocal(rr[:], rs[:])
            at_bf = sp.tile([24, 192], BF16, tag="atbf", name="atbf")
            nc.vector.tensor_tensor(
                at_bf[:].rearrange("p (h c) -> p h c", h=8),
                ex[:].rearrange("p (h c) -> p h c", h=8),
                rr[:, :, None].to_broadcast((24, 8, 24)), ALU.mult,
            )
            # block-diagonal attn (untransposed): BD2[c, vc] = A_bd[c, vc]
            bd2 = [sp.tile([128, 192], BF16, tag="bd20", name="bd20"),
                   sp.tile([64, 192], BF16, tag="bd21", name="bd21")]
            for bd in bd2:
                nc.gpsimd.memset(bd[:], 0.0)
            for h in range(5):
                nc.sync.dma_start(bd2[0][24 * h : 24 * h + 24, 24 * h : 24 * h + 24],
                                  at_bf[:, 24 * h : 24 * h + 24])
            nc.sync.dma_start(bd2[0][120:128, 120:144], at_bf[0:8, 120:144])
            nc.sync.dma_start(bd2[1][0:16, 120:144], at_bf[8:24, 120:144])
            nc.sync.dma_start(bd2[1][16:40, 144:168], at_bf[:, 144:168])
            nc.sync.dma_start(bd2[1][40:64, 168:192], at_bf[:, 168:192])

            # W2T[vc, o] = sum_c A_bd[c, vc] * wproj[o, c]  (fuses attn@v into proj)
            w2_sb = [sp.tile([128, C], BF16, tag="w20", name="w20"),
                     sp.tile([64, C], BF16, tag="w21", name="w21")]
            for j, (v0, v1) in enumerate(CHC):
                w2_ps = psg.tile([128, 192], F32, tag="gram2", name="w2ps")
                szj = v1 - v0
                nc.tensor.matmul(w2_ps[:szj], bd2[0][:, v0:v1], wp_sb[0][:],
                                 start=True, stop=False)
                nc.tensor.matmul(w2_ps[:szj], bd2[1][:, v0:v1], wp_sb[1][:],
                                 start=False, stop=True)
                nc.vector.tensor_copy(w2_sb[j][:szj], w2_ps[:szj])

            # =================== PHASE 2b ===================
            for t in range(NT):
                r0 = t * TH
                # v tile with 2-row halo: rows r0-2 .. r0+17 (20 rows)
                vt = [
                    wp.tile([128, 20, W], BF16, tag="xt0", name="xt0"),
                    wp.tile([64, 20, W], BF16, tag="xt1", name="xt1"),
                ]
                for k, (c0, c1) in enumerate(CHC):
                    if t == 0:
                        nc.gpsimd.memset(vt[k][:, 0:2, :], 0.0)
                        nc.sync.dma_start(vt[k][:, 2:20, :], v_hbm[c0:c1, 0:18, :])
                    elif t == NT - 1:
                        nc.gpsimd.memset(vt[k][:, 18:20, :], 0.0)
                        nc.sync.dma_start(vt[k][:, 0:18, :], v_hbm[c0:c1, r0 - 2 : 128, :])
                    else:
                        nc.sync.dma_start(vt[k][:], v_hbm[c0:c1, r0 - 2 : r0 + 18, :])

                # ---- out_proj = W2 @ v (attn@v fused into proj) ----
                pj = [wp.tile([128, TH, W], F32, tag="pj0", name="pj0"), wp.tile([64, TH, W], F32, tag="pj1", name="pj1")]
                ei = 1
                for m, (o0, o1) in enumerate(CHC):
                    msz = CSZC[m]
                    for nn in range(4):
                        pj_ps = ps3.tile([128, 512], F32, tag="dw", name="pjp")
                        for k in range(2):
                            nc.tensor.matmul(
                                pj_ps[:msz],
                                w2_sb[k][:, o0:o1],
                                vt[k][:, 2 + 4 * nn : 6 + 4 * nn, :],
                                start=(k == 0), stop=(k == 1),
                            )
                        _evac(nc, ei, pj[m][:, 4 * nn : 4 * nn + 4, :],
                              pj_ps[:msz].rearrange("p (r w) -> p r w", w=W))
                        ei += 1

                # ---- pos1 = gelu(dwconv(v, wpos1)) over 18 rows ----
                p1 = [wp.tile([128, 18, W], BF16, tag="qp0", name="qp0"), wp.tile([64, 18, W], BF16, tag="qp1", name="qp1")]
                NR1 = [(0, 4), (4, 8), (8, 12), (12, 16), (16, 18)]
                for m in range(2):
                    msz = CSZC[m]
                    lhsT_taps = [dp1_sb[:msz, m, tap, :msz] for tap in range(9)]
                    for (a, b) in NR1:
                        p1_ps = ps3.tile([128, 512], F32, tag="pre", name="p1p")
                        p3 = p1_ps[:msz, 0 : (b - a) * W].rearrange("p (r w) -> p r w", w=W)
                        _dw_matmuls(nc, p3, lhsT_taps, vt[m][:], row_off=1 + a, nrows=b - a)
                        nc.scalar.activation(p1[m][:, a:b, :], p3, AF.Gelu)
                # zero out-of-image rows of pos1 (pos2 pads with 0)
                if t == 0:
                    for m in range(2):
                        nc.gpsimd.memset(p1[m][:, 0:1, :], 0.0)
                if t == NT - 1:
                    for m in range(2):
                        nc.gpsimd.memset(p1[m][:, 17:18, :], 0.0)

                # ---- pos2 = dwconv(pos1, wpos2); final = proj + bproj + pos2 ----
                outt = [op.tile([128, TH, W], F32, tag="ot0", name="ot0"), op.tile([64, TH, W], F32, tag="ot1", name="ot1")]
                for m in range(2):
                    msz = CSZC[m]
                    lhsT_taps = [dp2_sb[:msz, m, tap, :msz] for tap in range(9)]
                    for nn in range(4):
                        p2_ps = ps3.tile([128, 512], F32, tag="dw", name="p2p")
                        p3 = p2_ps[:msz].rearrange("p (r w) -> p r w", w=W)
                        _dw_matmuls(nc, p3, lhsT_taps, p1[m][:], row_off=1 + 4 * nn, nrows=4)
                        nc.vector.scalar_tensor_tensor(
                            outt[m][:, 4 * nn : 4 * nn + 4, :],
                            p3,
                            bproj_sb[:msz, m : m + 1],
                            pj[m][:, 4 * nn : 4 * nn + 4, :],
                            ALU.add, ALU.add,
                        )
                for m, (c0, c1) in enumerate(CHC):
                    nc.sync.dma_start(out_d[c0:c1, r0 : r0 + TH, :], outt[m][:])

    nc.compile()
    return nc


_NC = None


def _get_nc():
    global _NC
    if _NC is None:
        _NC = build_kernel()
    return _NC


def prepare_in_maps(inputs):
    x = np.asarray(inputs["x"], dtype=np.float32)          # [8, 192, 128, 128]
    w_qkv = np.asarray(inputs["w_qkv"], dtype=np.float32)  # [576, 192]
    b_qkv = np.asarray(inputs["b_qkv"], dtype=np.float32)  # [576]
    w_dw = np.asarray(inputs["w_dw"], dtype=np.float32)    # [576, 1, 3, 3]
    b_dw = np.asarray(inputs["b_dw"], dtype=np.float32)    # [576]
    w_proj = np.asarray(inputs["w_proj"], dtype=np.float32)  # [192, 192]
    b_proj = np.asarray(inputs["b_proj"], dtype=np.float32)  # [192]
    w_pos1 = np.asarray(inputs["w_pos1"], dtype=np.float32)  # [192, 1, 3, 3]
    w_pos2 = np.asarray(inputs["w_pos2"], dtype=np.float32)  # [192, 1, 3, 3]
    temperature = np.asarray(inputs["temperature"], dtype=np.float32)  # [8,1,1]

    bf = ml_dtypes.bfloat16

    def diag_pack(wd, nchunk, chunks):
        d = np.zeros((nchunk, 9, 128, 128), dtype=np.float32)
        for m, (c0, c1) in enumerate(chunks):
            sz = c1 - c0
            for tap, (i, j) in enumerate(TAPS):
                d[m, tap, :sz, :sz] = np.diag(wd[c0:c1, 0, i, j])
        return np.ascontiguousarray(d.transpose(2, 0, 1, 3)).astype(bf)

    def pad_bias(b, nchunk, chunks):
        out = np.zeros((128, nchunk), dtype=np.float32)
        for m, (c0, c1) in enumerate(chunks):
            out[: c1 - c0, m] = b[c0:c1]
        return out

    def dq8_pack(wd):
        f8 = ml_dtypes.float8_e4m3
        d = np.zeros((3, 6, 2, 128, 128), dtype=np.float32)
        for m in range(3):
            c0 = 128 * m
            for p in range(6):
                if p < 3:
                    dx = p - 1
                    d[m, p, 0] = np.diag(wd[c0:c0 + 128, 0, 0, dx + 1]) * DW_SCALE
                    d[m, p, 1] = np.diag(wd[c0:c0 + 128, 0, 1, dx + 1]) * DW_SCALE
                else:
                    dx = p - 4
                    d[m, p, 0] = np.diag(wd[c0:c0 + 128, 0, 2, dx + 1]) * DW_SCALE
        return np.ascontiguousarray(d.transpose(3, 0, 1, 2, 4)).astype(f8)

    idmaskA = np.zeros((96, 384), dtype=np.float32)
    idmaskB = np.zeros((96, 192), dtype=np.float32)
    for q in range(2):
        for i in range(96):
            idmaskA[i, 192 * q + i] = 1.0
            idmaskB[i, 96 * q + i] = 1.0

    shared = {
        "wqkvT": np.ascontiguousarray(w_qkv.T).astype(bf),
        "wprojT": np.ascontiguousarray(w_proj.T).astype(bf),
        "dqkv": diag_pack(w_dw[384:], 2, [(0, 128), (128, 192)]),
        "dq8": dq8_pack(w_dw),
        "dpos1": diag_pack(w_pos1, 2, CHC),
        "dpos2": diag_pack(w_pos2, 2, CHC),
        "bqkv": pad_bias(b_qkv, 5, CH3),
        "bdw": pad_bias(b_dw, 5, CH3),
        "bproj": pad_bias(b_proj, 2, CHC),
        "temp": temperature.reshape(8, 1),
        "idmaskA": idmaskA,
        "idmaskB": idmaskB,
    }
    in_maps = [dict(shared, x=x[i].astype(bf)) for i in range(B)]

    return in_maps


def kernel(**inputs):
    in_maps = prepare_in_maps(inputs)
    nc = _get_nc()
    res = run_bass_kernel_spmd(nc, in_maps, core_ids=list(range(B)))
    out = np.stack([res.results[i]["out"] for i in range(B)], axis=0)
    return out.astype(np.float32)

